# revision 7
# baseline (speedup 1.0000x reference)
"""Trainium2 Bass kernel for BinaryLinearWscales.

Math:  out = x @ (wscale * sign(weight) + wbias).T
     = wscale_n * (x @ sign(weight).T)_tn + wbias_n * rowsum(x)_t

sign(weight) is exactly representable in bf16 (+-1), so we run the matmul
on the tensor engine in bf16.  To recover ~fp32 accuracy, x is split into
x_hi = bf16(x) and x_lo = bf16(x - x_hi); both accumulate into the same
PSUM tile (error ~1e-5 instead of ~2e-3 for plain bf16).

Sharding: tensor-parallel over DOUT: each of the 8 cores gets 512 rows of
weight/wscale/wbias and the full x; outputs are concatenated on the host
along the feature dim.

Per-core pipeline:
  - weights: DMA natural [n,k] f32 -> sign via bit trick (DVE) -> bf16
    -> PE transpose (bf16 via identity) -> PSUM -> ACT copy to SBUF
    (sw_T[kc] tiles [128k x 512n], cached for the whole kernel)
  - x: DMA natural [t,k] f32 (contiguous, 2MB chunks) -> PE transpose
    (f32 via identity) -> PSUM -> hi = ACT copy-cast to bf16,
    lo = DVE subtract(psum_f32, hi_bf16) -> bf16
  - matmul: psum_out[t128, n512] += x_hiT.T @ swT + x_loT.T @ swT over 32
    k-chunks; psum_xsum[t128,1] += x_hiT.T @ ones  (rowsum of x, used for
    the wbias term; 60-cycle N=1 matmuls, nearly free)
  - epilogue (DVE): out = psum_out * wscale_rep + wbias_rep * xsum
    (wscale/wbias replicated across partitions once via gpsimd
    partition_broadcast)
"""

import os
from contextlib import ExitStack

import numpy as np

P = 128

# full problem dims
B, S, DIN, DOUT = 2, 2048, 4096, 4096
N_CORES = 8
N_SHARD = DOUT // N_CORES  # 512


def build_body(ctx, tc, out_ap, x_ap, w_ap, wscale_ap, wbias_ap, two_pass=True):
    import concourse.bass as bass
    from concourse import mybir
    from concourse.bass import ts
    from concourse.masks import make_identity

    nc = tc.nc
    T, K = x_ap.shape
    N, K2 = w_ap.shape
    assert K == K2
    assert T % 256 == 0 and K % P == 0 and N % P == 0 and N <= 512
    KC = K // P  # k chunks
    NB = N // P  # weight row blocks
    TGRP = 256  # tokens per transpose group
    TB = TGRP // P  # 2
    NTG = T // TGRP

    f32 = mybir.dt.float32
    bf16 = mybir.dt.bfloat16
    u32 = mybir.dt.uint32
    Alu = mybir.AluOpType

    # ---------------- constants ----------------
    const = ctx.enter_context(tc.tile_pool(name="const", bufs=1))
    ident_f32 = const.tile([P, P], f32, name="ident_f32", tag="ident_f32")
    make_identity(nc, ident_f32)
    ident_bf16 = const.tile([P, P], bf16, name="ident_bf16", tag="ident_bf16")
    nc.vector.tensor_copy(ident_bf16[:], ident_f32[:])
    ones_col = const.tile([P, 1], bf16, name="ones_col", tag="ones_col")
    nc.vector.memset(ones_col[:], 1.0)

    # wscale / wbias replicated across all 128 partitions
    wsc_stage = const.tile([1, N], f32, name="wsc_stage", tag="wsc_stage")
    nc.sync.dma_start(wsc_stage[:], wscale_ap[:, :])
    wbi_stage = const.tile([1, N], f32, name="wbi_stage", tag="wbi_stage")
    nc.sync.dma_start(wbi_stage[:], wbias_ap[:, :])
    wscale_rep = const.tile([P, N], f32, name="wscale_rep", tag="wscale_rep")
    nc.gpsimd.partition_broadcast(wscale_rep[:], wsc_stage[:])
    wbias_rep = const.tile([P, N], f32, name="wbias_rep", tag="wbias_rep")
    nc.gpsimd.partition_broadcast(wbias_rep[:], wbi_stage[:])

    # ---------------- weight phase ----------------
    # sw_T[kc]: [128 k, N n] bf16 tiles of sign(w).T, cached for whole kernel
    swt_pool = ctx.enter_context(tc.tile_pool(name="swt", bufs=1))
    swT = [
        swt_pool.tile([P, N], bf16, name=f"swT{kc}", tag=f"swT{kc}")
        for kc in range(KC)
    ]
    with tc.tile_pool(name="wphase", bufs=1) as wpool, tc.tile_pool(
        name="wpsum", bufs=2, space="PSUM"
    ) as wpsum_pool:
        s_nats = []
        for nb in range(NB):
            w_nat = wpool.tile([P, K], f32, name=f"w_nat{nb}", tag="w_nat", bufs=2)
            nc.sync.dma_start(w_nat[:], w_ap[ts(nb, P), :])
            # sign via bit trick: (w & 0x80000000) | 0x3f800000  -> +-1.0f
            s_f32 = wpool.tile([P, K], f32, name=f"s_f32_{nb}", tag="s_f32", bufs=2)
            nc.vector.tensor_scalar(
                out=s_f32.bitcast(u32),
                in0=w_nat.bitcast(u32),
                scalar1=0x80000000,
                scalar2=0x3F800000,
                op0=Alu.bitwise_and,
                op1=Alu.bitwise_or,
            )
            s_nat = wpool.tile([P, K], bf16, name=f"s_nat{nb}", tag=f"s_nat{nb}")
            nc.scalar.copy(s_nat[:], s_f32[:])  # exact: values are +-1.0
            s_nats.append(s_nat)
        for kc in range(KC):
            pw = wpsum_pool.tile([P, N], bf16, name=f"pw{kc}", tag="pw")
            for nb in range(NB):
                nc.tensor.transpose(
                    pw[:, ts(nb, P)], s_nats[nb][:, ts(kc, P)], ident_bf16
                )
            nc.scalar.copy(swT[kc][:], pw[:])

    # ---------------- main phase ----------------
    xnat_pool = ctx.enter_context(tc.tile_pool(name="xnat", bufs=4))
    xsplit_pool = ctx.enter_context(tc.tile_pool(name="xsplit", bufs=36))
    opool = ctx.enter_context(tc.tile_pool(name="opool", bufs=3))
    psx_pool = ctx.enter_context(tc.tile_pool(name="psx", bufs=3, space="PSUM"))
    pox_pool = ctx.enter_context(tc.tile_pool(name="pox", bufs=2, space="PSUM"))
    pss_pool = ctx.enter_context(tc.tile_pool(name="pss", bufs=2, space="PSUM"))

    for tg in range(NTG):
        x_nats = []
        for tb in range(TB):
            x_nat = xnat_pool.tile([P, K], f32, name=f"x_nat_{tg}_{tb}", tag="x_nat")
            nc.sync.dma_start(x_nat[:], x_ap[ts(tg * TB + tb, P), :])
            x_nats.append(x_nat)
        xhis = []
        xlos = []
        for kc in range(KC):
            psx = psx_pool.tile([P, TGRP], f32, name=f"psx_{tg}_{kc}", tag="psx")
            for tb in range(TB):
                nc.tensor.transpose(
                    psx[:, ts(tb, P)], x_nats[tb][:, ts(kc, P)], ident_f32
                )
            xhi = xsplit_pool.tile([P, TGRP], bf16, name=f"xhi_{tg}_{kc}", tag="xhi")
            nc.scalar.copy(xhi[:], psx[:])
            xhis.append(xhi)
            if two_pass:
                xlo = xsplit_pool.tile(
                    [P, TGRP], bf16, name=f"xlo_{tg}_{kc}", tag="xlo"
                )
                nc.vector.tensor_sub(xlo[:], psx[:], xhi[:])
                xlos.append(xlo)

        for ot in range(TB):
            psum_o = pox_pool.tile([P, 512], f32, name=f"po_{tg}_{ot}", tag="po")[
                :, :N
            ]
            psum_s = pss_pool.tile([P, 8], f32, name=f"ps_{tg}_{ot}", tag="ps")
            for kc in range(KC):
                lhs_hi = xhis[kc][:, ts(ot, P)]
                nc.tensor.matmul(
                    psum_o,
                    lhs_hi,
                    swT[kc][:],
                    start=(kc == 0),
                    stop=(not two_pass and kc == KC - 1),
                )
                nc.tensor.matmul(
                    psum_s[:, 0:1],
                    lhs_hi,
                    ones_col[:],
                    start=(kc == 0),
                    stop=(kc == KC - 1),
                )
                if two_pass:
                    lhs_lo = xlos[kc][:, ts(ot, P)]
                    nc.tensor.matmul(
                        psum_o,
                        lhs_lo,
                        swT[kc][:],
                        start=False,
                        stop=(kc == KC - 1),
                    )
            out_sb = opool.tile([P, N], f32, name=f"out_sb_{tg}_{ot}", tag="out_sb")
            nc.vector.tensor_mul(out_sb[:], psum_o, wscale_rep[:])
            nc.vector.scalar_tensor_tensor(
                out=out_sb[:],
                in0=wbias_rep[:],
                scalar=psum_s[:, 0:1],
                in1=out_sb[:],
                op0=Alu.mult,
                op1=Alu.add,
            )
            nc.sync.dma_start(out_ap[ts(tg * TB + ot, P), :], out_sb[:])


def build_nc(T, K, N, two_pass=True):
    import concourse.tile as tile
    from concourse import bacc, mybir

    nc = bacc.Bacc(
        "TRN2",
        target_bir_lowering=False,
        debug=False,
        enable_asserts=False,
    )
    f32 = mybir.dt.float32
    x_t = nc.dram_tensor("x", [T, K], f32, kind="ExternalInput")
    w_t = nc.dram_tensor("w", [N, K], f32, kind="ExternalInput")
    wsc_t = nc.dram_tensor("wscale", [1, N], f32, kind="ExternalInput")
    wbi_t = nc.dram_tensor("wbias", [1, N], f32, kind="ExternalInput")
    out_t = nc.dram_tensor("out", [T, N], f32, kind="ExternalOutput")

    with tile.TileContext(nc) as tc:
        with ExitStack() as ctx:
            build_body(
                ctx,
                tc,
                out_t.ap(),
                x_t.ap(),
                w_t.ap(),
                wsc_t.ap(),
                wbi_t.ap(),
                two_pass=two_pass,
            )
    nc.compile()
    return nc


_NC_CACHE = {}
_LAST_RESULT = None


def _get_nc(T, K, N, two_pass):
    key = (T, K, N, two_pass)
    if key not in _NC_CACHE:
        _NC_CACHE[key] = build_nc(T, K, N, two_pass)
    return _NC_CACHE[key]


def _make_in_maps(inputs):
    x = inputs["x"] if "x" in inputs else inputs.get("x")
    weight = inputs["weight"]
    wscale = inputs["wscale"]
    wbias = inputs["wbias"]
    x = np.ascontiguousarray(np.asarray(x, dtype=np.float32).reshape(B * S, DIN))
    weight = np.asarray(weight, dtype=np.float32)
    wscale = np.asarray(wscale, dtype=np.float32).reshape(-1)
    wbias = np.asarray(wbias, dtype=np.float32).reshape(-1)
    in_maps = []
    for c in range(N_CORES):
        sl = slice(c * N_SHARD, (c + 1) * N_SHARD)
        in_maps.append(
            {
                "x": x,
                "w": np.ascontiguousarray(weight[sl]),
                "wscale": np.ascontiguousarray(wscale[sl]).reshape(1, N_SHARD),
                "wbias": np.ascontiguousarray(wbias[sl]).reshape(1, N_SHARD),
            }
        )
    return in_maps


def kernel(x, weight, wscale, wbias):
    from concourse.bass_utils import run_bass_kernel_spmd

    two_pass = os.environ.get("KERNEL_ONE_PASS", "0") != "1"
    nc = _get_nc(B * S, DIN, N_SHARD, two_pass)
    in_maps = _make_in_maps(
        {"x": x, "weight": weight, "wscale": wscale, "wbias": wbias}
    )

    trace = os.environ.get("KERNEL_TRACE", "0") == "1"
    res = run_bass_kernel_spmd(
        nc, in_maps, core_ids=list(range(N_CORES)), trace=trace
    )
    global _LAST_RESULT
    _LAST_RESULT = res
    if trace and res.exec_time_ns is not None:
        print(f"HW exec time: {res.exec_time_ns} ns")
    outs = [res.results[c]["out"] for c in range(N_CORES)]
    full = np.concatenate(outs, axis=1)  # [T, DOUT]
    return full.reshape(B, S, DOUT).astype(np.float32)
